# revision 15
# baseline (speedup 1.0000x reference)
"""ExtractSearchWindows Trainium2 kernel (8 NeuronCores, Bass/Tile).

out[b, h, w, dy*cv+dx, ky*8+kx] = uint8(P[b, h+off+dy+ky, w+off+dx+kx])
with P = zero-pad(inputs[:, 0], 7) and off = 3 - search_range.

Strategy: the output (196.6 MB u8) is a pure byte-replication of a tiny
input, so the kernel is bound by the SBUF->HBM DMA fabric (~435 GB/s/core
= 16 SDMA engines x ~27 GiB/s).  Work is sharded over (b, h): each of the
8 cores produces 48 output rows.

Host prep (tiny): pad+cast the 0.5 MB input to u8 and lay out, per core,
a 1.6 MB array of byte-shifted sub-rows "S" such that every device-side
expansion copy becomes a 4-byte-aligned strided uint32 tensor_copy
(phase-decomposed over w mod 4).  Device per core: 3 tiles x 128
segments (segment = 40-pixel row chunk); per tile, strided u32 DVE
copies -> 4 MB contiguous DMA-out in final (w, d, t) byte order.
Pipeline: chunk 0 is emitted in three pixel-range groups so the first
out-DMA launches after ~1/3 of the chunk's copies, and chunk 1 in two
half-chunks so the out-DMA queue never runs dry while DVE builds it.
"""
import numpy as np

K = 8
MAX_SR = 3
B, H, W = 2, 192, 320
TP = MAX_SR + K // 2          # 7 pad per side
PW = W + 2 * TP               # 334
NCORES = 8
ROWS_PER_CORE = (B * H) // NCORES   # 48
WSEG = 40
NWSEG = W // WSEG             # 8
NSEG = ROWS_PER_CORE * NWSEG  # 384
NTILE = NSEG // 128           # 3
NCH = 2                       # w-chunks per segment
WCH = WSEG // NCH             # 20 pixels per chunk
NA = WCH // 4                 # 5

_PROG_CACHE = {}


def _geom(sr):
    cv = 2 * sr + 1
    off = MAX_SR - sr
    nv = cv - 1 + K                  # source rows per output row
    nu = 4 + cv - 1                  # shifted sub-rows: phi + dx
    nj = 4 * (WSEG // 4 - 1) + (K - 1) + 1  # sub-row bytes (covers all chunks)
    nj = (nj + 3) // 4 * 4                  # pad to mult of 4 -> 44
    return cv, off, nv, nu, nj


NJF = 12                      # bytes per sub-row in the fill-lite array


def _make_fill_host(s_core, nv, nu, nj):
    """s_core: [NSEG, nv*nu*nj] u8 -> [128, nv*nu*NJF] fill-lite slice."""
    return np.ascontiguousarray(
        s_core[:128].reshape(128, nv * nu, nj)[:, :, :NJF]).reshape(128, -1)


def _make_s_host(x, sr):
    """x: (B,1,H,W) f32 -> per-core list of [NSEG, nv*nu*nj] u8 arrays."""
    cv, off, nv, nu, nj = _geom(sr)
    P = np.pad(x[:, 0], ((0, 0), (TP, TP), (TP, TP))).astype(np.uint8)
    cores = []
    for c in range(NCORES):
        b = (c * ROWS_PER_CORE) // H
        h0 = (c * ROWS_PER_CORE) % H
        flat = np.ascontiguousarray(P[b]).reshape(-1)
        base = (h0 + off) * PW + off
        s5 = np.lib.stride_tricks.as_strided(
            flat[base:], shape=(ROWS_PER_CORE, NWSEG, nv, nu, nj),
            strides=(PW, WSEG, PW, 1, 1))
        cores.append(np.ascontiguousarray(s5).reshape(NSEG, nv * nu * nj))
    return cores


def _strip_const_memsets(nc):
    """Drop the unused const-AP Memset preamble (saves ~0.4 us of startup)."""
    import concourse.mybir as mybir
    entry = nc.main_func.blocks[0]
    keep = []
    for inst in entry.instructions:
        if isinstance(inst, mybir.InstMemset) and inst.outs and \
                str(inst.outs[0].memsetref).startswith("const-"):
            continue
        keep.append(inst)
    entry.instructions[:] = keep


def _build_program(sr):
    import concourse.bass as bass
    import concourse.bacc as bacc
    import concourse.mybir as mybir
    from concourse import tile

    cv, off, nv, nu, nj = _geom(sr)
    segb = nv * nu * nj
    segw = segb // 4
    out_seg_b = WSEG * cv * cv * K * K
    ch_b = out_seg_b // NCH
    ch_w = ch_b // 4
    d_i32 = cv * K * K // 4        # u32 per pixel per dy (= 80 for cv=5)
    pix_i32 = cv * cv * K * K // 4  # u32 per pixel (= 400 for cv=5)

    u8 = mybir.dt.uint8
    u32 = mybir.dt.uint32
    nc = bacc.Bacc("TRN2", debug=False)
    _strip_const_memsets(nc)
    s_in = nc.declare_dram_parameter("s_in", [NSEG, segb], u8, isOutput=False)
    fillb = nv * nu * NJF
    s_fill = nc.declare_dram_parameter("s_fill", [128, fillb], u8,
                                       isOutput=False)
    out = nc.declare_dram_parameter("out", [NSEG * out_seg_b], u8, isOutput=True)

    with tile.TileContext(nc) as tc:
        with tc.tile_pool(name="spool", bufs=1) as sp, \
             tc.tile_pool(name="tpool", bufs=3) as tp:
            # All of S stays resident (12.7 KB/partition).  Two DMAs: tile 0
            # first so compute starts early, then tiles 1..NTILE-1.
            S = sp.tile([128, NTILE * segb], u8)
            SF = sp.tile([128, fillb], u8)
            nc.sync.dma_start(SF[:, :], s_fill[0:128, :])
            nc.sync.dma_start(S[:, 0:segb], s_in[0:128, :])
            rest_src = bass.AP(s_in.ap().tensor, 128 * segb,
                               [[segb, 128], [128 * segb, NTILE - 1],
                                [1, segb]])
            rest_dst = bass.AP(S[:].tensor, segb,
                               [[NTILE * segb, 128], [segb, NTILE - 1],
                                [1, segb]])
            nc.sync.dma_start(rest_dst, rest_src)
            s32 = S[:].bitcast(u32)
            sf32 = SF[:].bitcast(u32)
            njf_w = NJF // 4

            def emit_fill(w0, w1, T, t32):
                """Pixels [w0,w1) of chunk (0,0) from the fill-lite tile."""
                for dy in range(cv):
                    for phi in range(4):
                        a_lo = -(-(w0 - phi) // 4)
                        a_hi = (w1 - 1 - phi) // 4
                        if a_hi < a_lo:
                            continue
                        an = a_hi - a_lo + 1
                        src = bass.AP(
                            sf32.tensor,
                            dy * (nu * njf_w) + phi * njf_w + a_lo,
                            [[nv * nu * njf_w, 128], [nu * njf_w, K],
                             [1, an], [njf_w, cv], [1, 2]])
                        dst = bass.AP(
                            t32.tensor,
                            (4 * a_lo + phi) * pix_i32 + dy * d_i32,
                            [[ch_w, 128], [2, K], [4 * pix_i32, an],
                             [K * K // 4, cv], [1, 2]])
                        nc.vector.tensor_copy(dst, src)
                gb0 = w0 * cv * cv * K * K
                gbn = (w1 - w0) * cv * cv * K * K
                dst_hbm = bass.AP(out.ap().tensor, gb0,
                                  [[out_seg_b, 128], [1, gbn]])
                nc.sync.dma_start(dst_hbm, T[0:128, gb0:gb0 + gbn])

            def emit(t, ch, w0, w1, T, t32):
                """Copy pixels [w0,w1) of chunk (t,ch) into T; DMA them out."""
                for dy in range(cv):
                    for phi in range(4):
                        a_lo = -(-(w0 - phi) // 4)      # ceil
                        a_hi = (w1 - 1 - phi) // 4
                        if a_hi < a_lo:
                            continue
                        an = a_hi - a_lo + 1
                        src = bass.AP(
                            s32.tensor,
                            t * segw + dy * (nu * nj // 4)
                            + phi * (nj // 4) + NA * ch + a_lo,
                            [[NTILE * segw, 128],
                             [nu * nj // 4, K],  # ky: next src row
                             [1, an],            # a: +4 bytes
                             [nj // 4, cv],      # dx: next sub-row
                             [1, 2]])            # kx pair
                        dst = bass.AP(
                            t32.tensor,
                            (4 * a_lo + phi) * pix_i32 + dy * d_i32,
                            [[ch_w, 128],
                             [2, K],             # ky: +8 bytes
                             [4 * pix_i32, an],  # a: +4 pixels
                             [K * K // 4, cv],   # dx: +64 bytes
                             [1, 2]])            # kx pair
                        nc.vector.tensor_copy(dst, src)
                gb0 = w0 * cv * cv * K * K
                gbn = (w1 - w0) * cv * cv * K * K
                dst_hbm = bass.AP(
                    out.ap().tensor,
                    (t * 128) * out_seg_b + ch * ch_b + gb0,
                    [[out_seg_b, 128], [1, gbn]])
                nc.sync.dma_start(dst_hbm, T[0:128, gb0:gb0 + gbn])

            # Pixel-range units per chunk index (t*NCH+ch): chunk 0 in three
            # groups (pipeline fill), chunk 1 in two halves (keeps the DMA
            # queue fed while DVE builds it), the rest whole.
            unit_plan = {0: [(0, 8), (8, 16), (16, 20)],
                         1: [(0, 10), (10, 20)]}
            for t in range(NTILE):
                for ch in range(NCH):
                    T = tp.tile([128, ch_b], u8)
                    t32 = T[:].bitcast(u32)
                    units = unit_plan.get(t * NCH + ch, [(0, WCH)])
                    for (w0, w1) in units:
                        if t == 0 and ch == 0 and w1 <= 8:
                            emit_fill(w0, w1, T, t32)
                        else:
                            emit(t, ch, w0, w1, T, t32)
    nc.compile()
    return nc


def _numpy_fallback(x, sr):
    cv, off, _, _, _ = _geom(sr)
    P = np.pad(x[:, 0], ((0, 0), (TP, TP), (TP, TP))).astype(np.uint8)
    out = np.empty((B, H, W, cv * cv, K * K), np.uint8)
    for dy in range(cv):
        for dx in range(cv):
            for ky in range(K):
                for kx in range(K):
                    out[:, :, :, dy * cv + dx, ky * K + kx] = \
                        P[:, off + dy + ky:off + dy + ky + H,
                          off + dx + kx:off + dx + kx + W]
    return out


def kernel(inputs, search_range):
    from concourse.bass_utils import run_bass_kernel_spmd

    x = np.asarray(inputs, dtype=np.float32)
    sr = int(np.asarray(search_range))
    if sr != 2 or x.shape != (B, 1, H, W):
        return _numpy_fallback(x, sr)

    cv = 2 * sr + 1
    if sr not in _PROG_CACHE:
        _PROG_CACHE[sr] = _build_program(sr)
    nc = _PROG_CACHE[sr]

    _, _, nv, nu, nj = _geom(sr)
    s_cores = _make_s_host(x, sr)
    res = run_bass_kernel_spmd(
        nc, [{"s_in": s, "s_fill": _make_fill_host(s, nv, nu, nj)}
             for s in s_cores], list(range(NCORES)))
    outs = [np.asarray(res.results[c]["out"]) for c in range(NCORES)]
    return np.concatenate(outs).reshape(B, H, W, cv * cv, K * K)


# revision 16
# speedup vs baseline: 1.0837x; 1.0837x over previous
"""ExtractSearchWindows Trainium2 kernel (8 NeuronCores, Bass/Tile).

out[b, h, w, dy*cv+dx, ky*8+kx] = uint8(P[b, h+off+dy+ky, w+off+dx+kx])
with P = zero-pad(inputs[:, 0], 7) and off = 3 - search_range.

Strategy: the output (196.6 MB u8) is a pure byte-replication of a tiny
input, so the kernel is bound by the SBUF->HBM DMA fabric (~435 GB/s/core
= 16 SDMA engines x ~27 GiB/s).  Work is sharded over (b, h): each of the
8 cores produces 48 output rows.

Host prep (tiny): pad+cast the 0.5 MB input to u8 and lay out, per core,
a 1.6 MB array of byte-shifted sub-rows "S" such that every device-side
expansion copy becomes a 4-byte-aligned strided uint32 tensor_copy
(phase-decomposed over w mod 4).  Device per core: 3 tiles x 128
segments (segment = 40-pixel row chunk); per tile, strided u32 DVE
copies -> 4 MB contiguous DMA-out in final (w, d, t) byte order.
Pipeline: chunk 0 is emitted in three pixel-range groups so the first
out-DMA launches after ~1/3 of the chunk's copies, and chunk 1 in two
half-chunks so the out-DMA queue never runs dry while DVE builds it.
"""
import numpy as np

K = 8
MAX_SR = 3
B, H, W = 2, 192, 320
TP = MAX_SR + K // 2          # 7 pad per side
PW = W + 2 * TP               # 334
NCORES = 8
ROWS_PER_CORE = (B * H) // NCORES   # 48
WSEG = 40
NWSEG = W // WSEG             # 8
NSEG = ROWS_PER_CORE * NWSEG  # 384
NTILE = NSEG // 128           # 3
NCH = 2                       # w-chunks per segment
WCH = WSEG // NCH             # 20 pixels per chunk
NA = WCH // 4                 # 5

_PROG_CACHE = {}


def _geom(sr):
    cv = 2 * sr + 1
    off = MAX_SR - sr
    nv = cv - 1 + K                  # source rows per output row
    nu = 4 + cv - 1                  # shifted sub-rows: phi + dx
    nj = 4 * (WSEG // 4 - 1) + (K - 1) + 1  # sub-row bytes (covers all chunks)
    nj = (nj + 3) // 4 * 4                  # pad to mult of 4 -> 44
    return cv, off, nv, nu, nj


def _make_s_host(x, sr):
    """x: (B,1,H,W) f32 -> per-core list of [NSEG, nv*nu*nj] u8 arrays."""
    cv, off, nv, nu, nj = _geom(sr)
    P = np.pad(x[:, 0], ((0, 0), (TP, TP), (TP, TP))).astype(np.uint8)
    cores = []
    for c in range(NCORES):
        b = (c * ROWS_PER_CORE) // H
        h0 = (c * ROWS_PER_CORE) % H
        flat = np.ascontiguousarray(P[b]).reshape(-1)
        base = (h0 + off) * PW + off
        s5 = np.lib.stride_tricks.as_strided(
            flat[base:], shape=(ROWS_PER_CORE, NWSEG, nv, nu, nj),
            strides=(PW, WSEG, PW, 1, 1))
        cores.append(np.ascontiguousarray(s5).reshape(NSEG, nv * nu * nj))
    return cores


def _strip_const_memsets(nc):
    """Drop the unused const-AP Memset preamble (saves ~0.4 us of startup)."""
    import concourse.mybir as mybir
    entry = nc.main_func.blocks[0]
    keep = []
    for inst in entry.instructions:
        if isinstance(inst, mybir.InstMemset) and inst.outs and \
                str(inst.outs[0].memsetref).startswith("const-"):
            continue
        keep.append(inst)
    entry.instructions[:] = keep


def _build_program(sr):
    import concourse.bass as bass
    import concourse.bacc as bacc
    import concourse.mybir as mybir
    from concourse import tile

    cv, off, nv, nu, nj = _geom(sr)
    segb = nv * nu * nj
    segw = segb // 4
    out_seg_b = WSEG * cv * cv * K * K
    ch_b = out_seg_b // NCH
    ch_w = ch_b // 4
    d_i32 = cv * K * K // 4        # u32 per pixel per dy (= 80 for cv=5)
    pix_i32 = cv * cv * K * K // 4  # u32 per pixel (= 400 for cv=5)

    u8 = mybir.dt.uint8
    u32 = mybir.dt.uint32
    nc = bacc.Bacc("TRN2", debug=False)
    _strip_const_memsets(nc)
    s_in = nc.declare_dram_parameter("s_in", [NSEG, segb], u8, isOutput=False)
    out = nc.declare_dram_parameter("out", [NSEG * out_seg_b], u8, isOutput=True)

    with tile.TileContext(nc) as tc:
        with tc.tile_pool(name="spool", bufs=1) as sp, \
             tc.tile_pool(name="tpool", bufs=3) as tp:
            # All of S stays resident (12.7 KB/partition).  Two DMAs: tile 0
            # first so compute starts early, then tiles 1..NTILE-1.
            S = sp.tile([128, NTILE * segb], u8)
            nc.sync.dma_start(S[:, 0:segb], s_in[0:128, :])
            rest_src = bass.AP(s_in.ap().tensor, 128 * segb,
                               [[segb, 128], [128 * segb, NTILE - 1],
                                [1, segb]])
            rest_dst = bass.AP(S[:].tensor, segb,
                               [[NTILE * segb, 128], [segb, NTILE - 1],
                                [1, segb]])
            nc.sync.dma_start(rest_dst, rest_src)
            s32 = S[:].bitcast(u32)

            def emit(t, ch, w0, w1, T, t32):
                """Copy pixels [w0,w1) of chunk (t,ch) into T; DMA them out."""
                for dy in range(cv):
                    for phi in range(4):
                        a_lo = -(-(w0 - phi) // 4)      # ceil
                        a_hi = (w1 - 1 - phi) // 4
                        if a_hi < a_lo:
                            continue
                        an = a_hi - a_lo + 1
                        src = bass.AP(
                            s32.tensor,
                            t * segw + dy * (nu * nj // 4)
                            + phi * (nj // 4) + NA * ch + a_lo,
                            [[NTILE * segw, 128],
                             [nu * nj // 4, K],  # ky: next src row
                             [1, an],            # a: +4 bytes
                             [nj // 4, cv],      # dx: next sub-row
                             [1, 2]])            # kx pair
                        dst = bass.AP(
                            t32.tensor,
                            (4 * a_lo + phi) * pix_i32 + dy * d_i32,
                            [[ch_w, 128],
                             [2, K],             # ky: +8 bytes
                             [4 * pix_i32, an],  # a: +4 pixels
                             [K * K // 4, cv],   # dx: +64 bytes
                             [1, 2]])            # kx pair
                        nc.vector.tensor_copy(dst, src)
                gb0 = w0 * cv * cv * K * K
                gbn = (w1 - w0) * cv * cv * K * K
                dst_hbm = bass.AP(
                    out.ap().tensor,
                    (t * 128) * out_seg_b + ch * ch_b + gb0,
                    [[out_seg_b, 128], [1, gbn]])
                nc.sync.dma_start(dst_hbm, T[0:128, gb0:gb0 + gbn])

            # Pixel-range units per chunk index (t*NCH+ch): chunk 0 in three
            # groups (pipeline fill), chunk 1 in two halves (keeps the DMA
            # queue fed while DVE builds it), the rest whole.
            unit_plan = {0: [(0, 8), (8, 16), (16, 20)],
                         1: [(0, 10), (10, 20)]}
            for t in range(NTILE):
                for ch in range(NCH):
                    T = tp.tile([128, ch_b], u8)
                    t32 = T[:].bitcast(u32)
                    units = unit_plan.get(t * NCH + ch, [(0, WCH)])
                    for (w0, w1) in units:
                        emit(t, ch, w0, w1, T, t32)
    nc.compile()
    return nc


def _numpy_fallback(x, sr):
    cv, off, _, _, _ = _geom(sr)
    P = np.pad(x[:, 0], ((0, 0), (TP, TP), (TP, TP))).astype(np.uint8)
    out = np.empty((B, H, W, cv * cv, K * K), np.uint8)
    for dy in range(cv):
        for dx in range(cv):
            for ky in range(K):
                for kx in range(K):
                    out[:, :, :, dy * cv + dx, ky * K + kx] = \
                        P[:, off + dy + ky:off + dy + ky + H,
                          off + dx + kx:off + dx + kx + W]
    return out


def kernel(inputs, search_range):
    from concourse.bass_utils import run_bass_kernel_spmd

    x = np.asarray(inputs, dtype=np.float32)
    sr = int(np.asarray(search_range))
    if sr != 2 or x.shape != (B, 1, H, W):
        return _numpy_fallback(x, sr)

    cv = 2 * sr + 1
    if sr not in _PROG_CACHE:
        _PROG_CACHE[sr] = _build_program(sr)
    nc = _PROG_CACHE[sr]

    s_cores = _make_s_host(x, sr)
    res = run_bass_kernel_spmd(
        nc, [{"s_in": s} for s in s_cores], list(range(NCORES)))
    outs = [np.asarray(res.results[c]["out"]) for c in range(NCORES)]
    return np.concatenate(outs).reshape(B, H, W, cv * cv, K * K)
